# revision 3
# baseline (speedup 1.0000x reference)
"""Depthwise conv (5x5 + 51x51, 'same' padding) on 8 Trainium2 NeuronCores.

Algorithm
---------
Both convs are centered with 'same' padding, so
    dw5(x, w5) + dw51(x, w51) == dw51(x, w_eff),
    w_eff = w51 + center_embed(w5).

The single 51x51 depthwise conv is mapped onto the TensorEngine as a
column-conv (along H) expressed as a banded matmul, accumulated over the
51 kernel columns j in PSUM:

    out[c,h,w] = sum_j sum_r  S_cj[r,h] * xpad[c, r, w+j]
    S_cj[r,h]  = w_eff[c, r-h, j]  (0 <= r-h < 51, else 0)

Per (channel, image-group) this is 51 accumulating matmuls with
K=114 (padded height), M=64 (output rows), N=512 (8 images x 64 cols).

Sharding: channels (192 = 8 cores x 24). Host precomputes zero-padded
images and band matrices in numpy; device does only DMA + matmul + copy.
"""

import os
import sys

import numpy as np

for _p in ("/opt/trn_rl_repo", "/opt/pypackages"):
    if _p not in sys.path and os.path.isdir(_p):
        sys.path.append(_p)

import concourse.bacc as bacc
import concourse.bass as bass
import concourse.mybir as mybir
from concourse import bass_utils, tile

B, C, H, W = 32, 192, 64, 64
K = 51
PAD = K // 2
HE = H + 2 * PAD          # 114
N_CORES = 8
CPC = C // N_CORES        # 24 channels per core
G = 4                     # image groups per core
IPG = B // G              # 8 images per group

_MM_DT = mybir.dt.float32r
_F32 = mybir.dt.float32

# "v1": one 64-row matmul chain per (channel, group).
# "v2": two groups paired into col-groups 0-1 / 2-3 of the PE array
#        (concurrent 64-col tiles, ~2x PE throughput if HW overlaps).
VARIANT = os.environ.get("DW_VARIANT", "v2")

_CACHE = {}


def _build_program():
    """Build the (per-core identical) Bass program once."""
    nc = bacc.Bacc("TRN2", target_bir_lowering=False, debug=False,
                   enable_partition_id=False)

    xpad_d = nc.dram_tensor("xpad", [CPC, G, HE, IPG, HE], _MM_DT,
                            kind="ExternalInput")
    bands_d = nc.dram_tensor("bands", [CPC, HE, K, H], _MM_DT,
                             kind="ExternalInput")
    if VARIANT == "v2":
        out_d = nc.dram_tensor("out", [CPC, G // 2, 2 * H, IPG, W], _F32,
                               kind="ExternalOutput")
    else:
        out_d = nc.dram_tensor("out", [CPC, G, H, IPG, W], _F32,
                               kind="ExternalOutput")
    xpad_ap = xpad_d.ap()
    bands_ap = bands_d.ap()
    out_ap = out_d.ap()

    with tile.TileContext(nc) as tc:
        with (
            tc.tile_pool(name="bands", bufs=2) as bpool,
            tc.tile_pool(name="xin", bufs=4) as xpool,
            tc.tile_pool(name="psum", bufs=8, space="PSUM") as ppool,
            tc.tile_pool(name="oout", bufs=4) as opool,
        ):
            for c in range(CPC):
                bt = bpool.tile([HE, K, H], _MM_DT)
                nc.sync.dma_start(bt[:], bands_ap[c])
                if VARIANT == "v2":
                    for t in range(G // 2):
                        xta = xpool.tile([HE, IPG, HE], _MM_DT, tag="xt")
                        xtb = xpool.tile([HE, IPG, HE], _MM_DT, tag="xt")
                        nc.sync.dma_start(xta[:], xpad_ap[c, 2 * t])
                        nc.sync.dma_start(xtb[:], xpad_ap[c, 2 * t + 1])
                        ps = ppool.tile([2 * H, IPG, W], _F32)
                        for j in range(K):
                            nc.tensor.matmul(
                                ps[0:H], bt[:, j, :], xta[:, :, j:j + W],
                                start=(j == 0), stop=(j == K - 1),
                            )
                            nc.tensor.matmul(
                                ps[H:2 * H], bt[:, j, :], xtb[:, :, j:j + W],
                                start=(j == 0), stop=(j == K - 1),
                            )
                        ot = opool.tile([2 * H, IPG, W], _F32)
                        nc.vector.tensor_copy(ot[:], ps[:])
                        nc.sync.dma_start(out_ap[c, t], ot[:])
                else:
                    for g in range(G):
                        xt = xpool.tile([HE, IPG, HE], _MM_DT, tag="xt")
                        nc.sync.dma_start(xt[:], xpad_ap[c, g])
                        ps = ppool.tile([H, IPG, W], _F32)
                        for j in range(K):
                            nc.tensor.matmul(
                                ps[:],
                                bt[:, j, :],
                                xt[:, :, j:j + W],
                                start=(j == 0),
                                stop=(j == K - 1),
                            )
                        ot = opool.tile([H, IPG, W], _F32)
                        nc.vector.tensor_copy(ot[:], ps[:])
                        nc.sync.dma_start(out_ap[c, g], ot[:])

    nc.compile()
    return nc


def _prepare_inputs(x, w5, w51):
    """Numpy preprocessing: merged kernel, padded images, band matrices.

    Returns in_maps: list of {"xpad": ..., "bands": ...} per core.
    """
    x = np.ascontiguousarray(np.asarray(x, dtype=np.float32))
    w5 = np.asarray(w5, dtype=np.float32)
    w51 = np.asarray(w51, dtype=np.float32)

    w_eff = w51[:, 0].copy()                      # [C, 51, 51]
    c0 = (K - 5) // 2                             # 23
    w_eff[:, c0:c0 + 5, c0:c0 + 5] += w5[:, 0]

    xpad = np.zeros((B, C, HE, HE), np.float32)
    xpad[:, :, PAD:PAD + H, PAD:PAD + W] = x

    bands = np.zeros((C, HE, K, H), np.float32)
    for h in range(H):
        bands[:, h:h + K, :, h] = w_eff

    in_maps = []
    for k in range(N_CORES):
        chs = slice(k * CPC, (k + 1) * CPC)
        # [B, CPC, HE, HE] -> [CPC, G, HE, IPG, HE]
        xp = (
            xpad[:, chs]
            .reshape(G, IPG, CPC, HE, HE)
            .transpose(2, 0, 3, 1, 4)
        )
        in_maps.append({
            "xpad": np.ascontiguousarray(xp),
            "bands": np.ascontiguousarray(bands[chs]),
        })
    return in_maps


def _assemble_output(results):
    y = np.empty((B, C, H, W), np.float32)
    for k in range(N_CORES):
        o = results[k]["out"].reshape(CPC, G, H, IPG, W)
        # v2 layout [CPC, G//2, 2, H, IPG, W] reshapes identically:
        # (t, s) fuse to g = 2t + s in order.
        # -> [G, IPG, CPC, H, W] -> [B, CPC, H, W]
        y[:, k * CPC:(k + 1) * CPC] = (
            o.transpose(1, 3, 0, 2, 4).reshape(B, CPC, H, W)
        )
    return y


def kernel(x, w5, w51):
    if "nc" not in _CACHE:
        _CACHE["nc"] = _build_program()
    nc = _CACHE["nc"]
    in_maps = _prepare_inputs(x, w5, w51)
    res = bass_utils.run_bass_kernel_spmd(nc, in_maps,
                                          core_ids=list(range(N_CORES)))
    return _assemble_output(res.results)


# revision 7
# speedup vs baseline: 94.3776x; 94.3776x over previous
"""Depthwise conv (5x5 + 51x51, 'same' padding) on 8 Trainium2 NeuronCores.

Algorithm
---------
Both convs are centered with 'same' padding, so
    dw5(x, w5) + dw51(x, w51) == dw51(x, w_eff),
    w_eff = w51 + center_embed(w5).

The single 51x51 depthwise conv is mapped onto the TensorEngine as a
column-conv (along H) expressed as a banded matmul, accumulated over the
51 kernel columns j in PSUM:

    out[c,h,w] = sum_j sum_r  S_cj[r,h] * xpad[c, r, w+j]
    S_cj[r,h]  = w_eff[c, r-h, j]  (0 <= r-h < 51, else 0)

Per (channel, image-group) this is 51 accumulating matmuls with
K=114 (padded height), M=64 (output rows), N=512 (8 images x 64 cols).

Sharding: channels (192 = 8 cores x 24). Host precomputes zero-padded
images and band matrices in numpy; device does only DMA + matmul + copy.
"""

import os
import sys

import numpy as np

for _p in ("/opt/trn_rl_repo", "/opt/pypackages"):
    if _p not in sys.path and os.path.isdir(_p):
        sys.path.append(_p)

import concourse.bacc as bacc
import concourse.bass as bass
import concourse.mybir as mybir
from concourse import bass_utils, tile

B, C, H, W = 32, 192, 64, 64
K = 51
PAD = K // 2
HE = H + 2 * PAD          # 114
N_CORES = 8
CPC = C // N_CORES        # 24 channels per core
G = 4                     # image groups per core
IPG = B // G              # 8 images per group

_F32 = mybir.dt.float32
_DTYPES = {
    "fp32r": (mybir.dt.float32r, np.float32),
    "fp32": (mybir.dt.float32, np.float32),
    "fp16": (mybir.dt.float16, np.float16),
    "bf16": (mybir.dt.bfloat16, None),  # np dtype resolved lazily
}

# "v1": one 64-row matmul chain per (channel, group).
# "v2": two groups paired into col-groups 0-1 / 2-3 of the PE array
#        (concurrent 64-col tiles, ~2x PE throughput; HW-validated).
# fp16 operands: PE streams 16-bit moving operands at 2 elem/cycle and
# accumulates in fp32 PSUM; measured end-to-end rel err 3.6e-4 vs the
# fp64 reference (fp32r/v1 fallback measured 1.8e-4 at ~3x the time).
VARIANT = os.environ.get("DW_VARIANT", "v2")
DTYPE = os.environ.get("DW_DTYPE", "fp16")

_CACHE = {}


def _np_dt(name):
    if name == "bf16":
        import ml_dtypes
        return np.dtype(ml_dtypes.bfloat16)
    return _DTYPES[name][1]


def _build_program(variant=None, repeat=1, dtype=None):
    """Build the (per-core identical) Bass program once.

    repeat>1 re-runs the whole compute loop (same IO) for slope timing.
    """
    if variant is None:
        variant = VARIANT
    _MM_DT = _DTYPES[dtype or DTYPE][0]
    nc = bacc.Bacc("TRN2", target_bir_lowering=False, debug=False,
                   enable_partition_id=False)

    xpad_d = nc.dram_tensor("xpad", [CPC, G, HE, IPG, HE], _MM_DT,
                            kind="ExternalInput")
    bands_d = nc.dram_tensor("bands", [CPC, HE, K, H], _MM_DT,
                             kind="ExternalInput")
    if variant == "v2":
        out_d = nc.dram_tensor("out", [CPC, G // 2, 2 * H, IPG, W], _F32,
                               kind="ExternalOutput")
    else:
        out_d = nc.dram_tensor("out", [CPC, G, H, IPG, W], _F32,
                               kind="ExternalOutput")
    xpad_ap = xpad_d.ap()
    bands_ap = bands_d.ap()
    out_ap = out_d.ap()

    with tile.TileContext(nc) as tc:
        with (
            tc.tile_pool(name="bands", bufs=2) as bpool,
            tc.tile_pool(name="xin", bufs=4) as xpool,
            tc.tile_pool(name="psum", bufs=8, space="PSUM") as ppool,
            tc.tile_pool(name="oout", bufs=4) as opool,
        ):
            for _rep in range(repeat):
                for c in range(CPC):
                    bt = bpool.tile([HE, K, H], _MM_DT, tag="bt")
                    nc.sync.dma_start(bt[:], bands_ap[c])
                    if variant == "v2":
                        for t in range(G // 2):
                            xta = xpool.tile([HE, IPG, HE], _MM_DT, tag="xt")
                            xtb = xpool.tile([HE, IPG, HE], _MM_DT, tag="xt")
                            nc.sync.dma_start(xta[:], xpad_ap[c, 2 * t])
                            nc.sync.dma_start(xtb[:], xpad_ap[c, 2 * t + 1])
                            ps = ppool.tile([2 * H, IPG, W], _F32, tag="ps")
                            for j in range(K):
                                nc.tensor.matmul(
                                    ps[0:H], bt[:, j, :], xta[:, :, j:j + W],
                                    start=(j == 0), stop=(j == K - 1),
                                    skip_group_check=True,
                                )
                                nc.tensor.matmul(
                                    ps[H:2 * H], bt[:, j, :], xtb[:, :, j:j + W],
                                    start=(j == 0), stop=(j == K - 1),
                                    skip_group_check=True,
                                )
                            ot = opool.tile([2 * H, IPG, W], _F32, tag="ot")
                            nc.vector.tensor_copy(ot[:], ps[:])
                            nc.sync.dma_start(out_ap[c, t], ot[:])
                    else:
                        for g in range(G):
                            xt = xpool.tile([HE, IPG, HE], _MM_DT, tag="xt")
                            nc.sync.dma_start(xt[:], xpad_ap[c, g])
                            ps = ppool.tile([H, IPG, W], _F32, tag="ps")
                            for j in range(K):
                                nc.tensor.matmul(
                                    ps[:],
                                    bt[:, j, :],
                                    xt[:, :, j:j + W],
                                    start=(j == 0),
                                    stop=(j == K - 1),
                                )
                            ot = opool.tile([H, IPG, W], _F32, tag="ot")
                            nc.vector.tensor_copy(ot[:], ps[:])
                            nc.sync.dma_start(out_ap[c, g], ot[:])

    nc.compile()
    return nc


def _prepare_inputs(x, w5, w51, dtype=None):
    """Numpy preprocessing: merged kernel, padded images, band matrices.

    Returns in_maps: list of {"xpad": ..., "bands": ...} per core.
    """
    npdt = _np_dt(dtype or DTYPE)
    x = np.ascontiguousarray(np.asarray(x, dtype=np.float32))
    w5 = np.asarray(w5, dtype=np.float32)
    w51 = np.asarray(w51, dtype=np.float32)

    w_eff = w51[:, 0].copy()                      # [C, 51, 51]
    c0 = (K - 5) // 2                             # 23
    w_eff[:, c0:c0 + 5, c0:c0 + 5] += w5[:, 0]

    xpad = np.zeros((B, C, HE, HE), np.float32)
    xpad[:, :, PAD:PAD + H, PAD:PAD + W] = x
    xpad = xpad.astype(npdt)

    bands = np.zeros((C, HE, K, H), np.float32)
    for h in range(H):
        bands[:, h:h + K, :, h] = w_eff
    bands = bands.astype(npdt)

    in_maps = []
    for k in range(N_CORES):
        chs = slice(k * CPC, (k + 1) * CPC)
        # [B, CPC, HE, HE] -> [CPC, G, HE, IPG, HE]
        xp = (
            xpad[:, chs]
            .reshape(G, IPG, CPC, HE, HE)
            .transpose(2, 0, 3, 1, 4)
        )
        in_maps.append({
            "xpad": np.ascontiguousarray(xp),
            "bands": np.ascontiguousarray(bands[chs]),
        })
    return in_maps


def _assemble_output(results):
    y = np.empty((B, C, H, W), np.float32)
    for k in range(N_CORES):
        o = results[k]["out"].reshape(CPC, G, H, IPG, W)
        # v2 layout [CPC, G//2, 2, H, IPG, W] reshapes identically:
        # (t, s) fuse to g = 2t + s in order.
        # -> [G, IPG, CPC, H, W] -> [B, CPC, H, W]
        y[:, k * CPC:(k + 1) * CPC] = (
            o.transpose(1, 3, 0, 2, 4).reshape(B, CPC, H, W)
        )
    return y


def kernel(x, w5, w51):
    if "nc" not in _CACHE:
        _CACHE["nc"] = _build_program()
    nc = _CACHE["nc"]
    in_maps = _prepare_inputs(x, w5, w51)
    res = bass_utils.run_bass_kernel_spmd(nc, in_maps,
                                          core_ids=list(range(N_CORES)))
    return _assemble_output(res.results)


# revision 8
# speedup vs baseline: 172.2271x; 1.8249x over previous
"""Depthwise conv (5x5 + 51x51, 'same' padding) on 8 Trainium2 NeuronCores.

Algorithm
---------
Both convs are centered with 'same' padding, so
    dw5(x, w5) + dw51(x, w51) == dw51(x, w_eff),
    w_eff = w51 + center_embed(w5).

The single 51x51 depthwise conv is mapped onto the TensorEngine as a
column-conv (along H) expressed as a banded matmul, accumulated over the
51 kernel columns j in PSUM:

    out[c,h,w] = sum_j sum_r  S_cj[r,h] * xpad[c, r, w+j]
    S_cj[r,h]  = w_eff[c, r-h, j]  (0 <= r-h < 51, else 0)

Per (channel, image-group) this is 51 accumulating matmuls with
K=114 (padded height), M=64 (output rows), N=512 (8 images x 64 cols).

Sharding: channels (192 = 8 cores x 24). Host precomputes zero-padded
images and band matrices in numpy; device does only DMA + matmul + copy.
"""

import os
import sys

import numpy as np

for _p in ("/opt/trn_rl_repo", "/opt/pypackages"):
    if _p not in sys.path and os.path.isdir(_p):
        sys.path.append(_p)

import concourse.bacc as bacc
import concourse.bass as bass
import concourse.mybir as mybir
from concourse import bass_utils, tile

B, C, H, W = 32, 192, 64, 64
K = 51
PAD = K // 2
HE = H + 2 * PAD          # 114
N_CORES = 8
CPC = C // N_CORES        # 24 channels per core
G = 4                     # image groups per core
IPG = B // G              # 8 images per group

_F32 = mybir.dt.float32
_DTYPES = {
    "fp32r": (mybir.dt.float32r, np.float32),
    "fp32": (mybir.dt.float32, np.float32),
    "fp16": (mybir.dt.float16, np.float16),
    "bf16": (mybir.dt.bfloat16, None),  # np dtype resolved lazily
}

# "v1": one 64-row matmul chain per (channel, group).
# "v2": two groups paired into col-groups 0-1 / 2-3 of the PE array
#        (concurrent 64-col tiles, ~2x PE throughput; HW-validated).
# fp16 operands: PE streams 16-bit moving operands at 2 elem/cycle and
# accumulates in fp32 PSUM; measured end-to-end rel err 3.6e-4 vs the
# fp64 reference (fp32r/v1 fallback measured 1.8e-4 at ~3x the time).
VARIANT = os.environ.get("DW_VARIANT", "v2")
DTYPE = os.environ.get("DW_DTYPE", "fp16")

_CACHE = {}


def _np_dt(name):
    if name == "bf16":
        import ml_dtypes
        return np.dtype(ml_dtypes.bfloat16)
    return _DTYPES[name][1]


def _build_program(variant=None, repeat=1, dtype=None):
    """Build the (per-core identical) Bass program once.

    repeat>1 re-runs the whole compute loop (same IO) for slope timing.
    """
    if variant is None:
        variant = VARIANT
    _MM_DT = _DTYPES[dtype or DTYPE][0]
    nc = bacc.Bacc("TRN2", target_bir_lowering=False, debug=False,
                   enable_partition_id=False)

    xpad_d = nc.dram_tensor("xpad", [CPC, G, HE, IPG, HE], _MM_DT,
                            kind="ExternalInput")
    bands_d = nc.dram_tensor("bands", [CPC, HE, K, H], _MM_DT,
                             kind="ExternalInput")
    if variant == "v2":
        out_d = nc.dram_tensor("out", [CPC, G // 2, 2 * H, IPG, W], _F32,
                               kind="ExternalOutput")
    else:
        out_d = nc.dram_tensor("out", [CPC, G, H, IPG, W], _F32,
                               kind="ExternalOutput")
    xpad_ap = xpad_d.ap()
    bands_ap = bands_d.ap()
    out_ap = out_d.ap()

    with tile.TileContext(nc) as tc:
        with (
            tc.tile_pool(name="bands", bufs=2) as bpool,
            tc.tile_pool(name="xin", bufs=4) as xpool,
            tc.tile_pool(name="psum", bufs=8, space="PSUM") as ppool,
            tc.tile_pool(name="oout", bufs=4) as opool,
        ):
            for _rep in range(repeat):
                for c in range(CPC):
                    bt = bpool.tile([HE, K, H], _MM_DT, tag="bt")
                    nc.sync.dma_start(bt[:], bands_ap[c])
                    if variant == "v2":
                        for t in range(G // 2):
                            xta = xpool.tile([HE, IPG, HE], _MM_DT, tag="xt")
                            xtb = xpool.tile([HE, IPG, HE], _MM_DT, tag="xt")
                            nc.sync.dma_start(xta[:], xpad_ap[c, 2 * t])
                            nc.sync.dma_start(xtb[:], xpad_ap[c, 2 * t + 1])
                            ps = ppool.tile([2 * H, IPG, W], _F32, tag="ps")
                            for j in range(K):
                                nc.tensor.matmul(
                                    ps[0:H], bt[:, j, :], xta[:, :, j:j + W],
                                    start=(j == 0), stop=(j == K - 1),
                                    skip_group_check=True,
                                )
                                nc.tensor.matmul(
                                    ps[H:2 * H], bt[:, j, :], xtb[:, :, j:j + W],
                                    start=(j == 0), stop=(j == K - 1),
                                    skip_group_check=True,
                                )
                            ot = opool.tile([2 * H, IPG, W], _F32, tag="ot")
                            nc.vector.tensor_copy(ot[:], ps[:])
                            nc.sync.dma_start(out_ap[c, t], ot[:])
                    else:
                        for g in range(G):
                            xt = xpool.tile([HE, IPG, HE], _MM_DT, tag="xt")
                            nc.sync.dma_start(xt[:], xpad_ap[c, g])
                            ps = ppool.tile([H, IPG, W], _F32, tag="ps")
                            for j in range(K):
                                nc.tensor.matmul(
                                    ps[:],
                                    bt[:, j, :],
                                    xt[:, :, j:j + W],
                                    start=(j == 0),
                                    stop=(j == K - 1),
                                )
                            ot = opool.tile([H, IPG, W], _F32, tag="ot")
                            nc.vector.tensor_copy(ot[:], ps[:])
                            nc.sync.dma_start(out_ap[c, g], ot[:])

    nc.compile()
    return nc


def _prepare_inputs(x, w5, w51, dtype=None):
    """Numpy preprocessing: merged kernel, padded images, band matrices.

    Returns in_maps: list of {"xpad": ..., "bands": ...} per core.
    """
    npdt = _np_dt(dtype or DTYPE)
    x = np.asarray(x, dtype=np.float32)
    w5 = np.asarray(w5, dtype=np.float32)
    w51 = np.asarray(w51, dtype=np.float32)

    w_eff = w51[:, 0].copy()                      # [C, 51, 51]
    c0 = (K - 5) // 2                             # 23
    w_eff[:, c0:c0 + 5, c0:c0 + 5] += w5[:, 0]

    xpad = np.zeros((B, C, HE, HE), npdt)
    xpad[:, :, PAD:PAD + H, PAD:PAD + W] = x.astype(npdt)

    w_eff = w_eff.astype(npdt)
    bands = np.zeros((C, HE, K, H), npdt)
    for h in range(H):
        bands[:, h:h + K, :, h] = w_eff

    in_maps = []
    for k in range(N_CORES):
        chs = slice(k * CPC, (k + 1) * CPC)
        # [B, CPC, HE, HE] -> [CPC, G, HE, IPG, HE]
        xp = (
            xpad[:, chs]
            .reshape(G, IPG, CPC, HE, HE)
            .transpose(2, 0, 3, 1, 4)
        )
        in_maps.append({
            "xpad": np.ascontiguousarray(xp),
            "bands": np.ascontiguousarray(bands[chs]),
        })
    return in_maps


def _assemble_output(results):
    y = np.empty((B, C, H, W), np.float32)
    for k in range(N_CORES):
        o = results[k]["out"].reshape(CPC, G, H, IPG, W)
        # v2 layout [CPC, G//2, 2, H, IPG, W] reshapes identically:
        # (t, s) fuse to g = 2t + s in order.
        # -> [G, IPG, CPC, H, W] -> [B, CPC, H, W]
        y[:, k * CPC:(k + 1) * CPC] = (
            o.transpose(1, 3, 0, 2, 4).reshape(B, CPC, H, W)
        )
    return y


def kernel(x, w5, w51):
    if "nc" not in _CACHE:
        _CACHE["nc"] = _build_program()
    nc = _CACHE["nc"]
    in_maps = _prepare_inputs(x, w5, w51)
    res = bass_utils.run_bass_kernel_spmd(nc, in_maps,
                                          core_ids=list(range(N_CORES)))
    return _assemble_output(res.results)
